# revision 34
# baseline (speedup 1.0000x reference)
"""Two-layer GAT (PyG GATConv semantics) on 8 Trainium2 NeuronCores.

Strategy (graph/data parallel, per the sharding hint):
  - Nodes are sharded across 8 cores by contiguous ranges of 12500; each core
    owns its node shard plus all edges whose *dst* lands in the shard.
  - Per core, dst nodes are sorted by in-degree and packed into batches of
    128 (one node per SBUF partition). Each node's incoming edges occupy L
    slots along the free dim (L = per-batch max degree, shared across cores
    so one SPMD program serves all 8). The segment softmax / segment sum
    then become native free-dim vector reductions.
  - Per-edge src features are fetched with indirect DMA (row gather) from a
    replicated node-feature table. The LAYER-1 table is computed redundantly
    on every core (h = x@W1 from a replicated bf16 x input; private table,
    no collective — cheaper than an AllGather on the critical path, since
    collectives occupy the issuing gpsimd queue for their whole transfer).
    The LAYER-2 table is AllGathered in two uneven row-range chunks: a big
    chunk fired as soon as its rows are produced and a small tail chunk, to
    minimize gpsimd-queue occupancy and the inter-layer wait.
  - The gather is the bottleneck: each [128,1]-offset indirect DMA costs
    ~1us of serial GPSIMD descriptor-generation time (the only offset form
    the HW DGE lowers correctly), so runtime ~= #slot-columns x 1us.
    Self-loop edges are therefore excluded from the slabs (-6% columns);
    their softmax/message contribution is added from SBUF-resident per-node
    [h | a_src.h | a_dst.h] tiles kept from the layer's matmul phase.
  - Tables are bf16 (halves gather bandwidth); accumulations are fp32.

Host-side work is limited to integer index manipulation (sharding, sorting,
padding, building gather offset arrays) and pure relayouts (transposes /
block-diagonal placement of input tensors).
"""

import sys

for _p in ("/opt/trn_rl_repo",):
    if _p not in sys.path:
        sys.path.insert(0, _p)

import numpy as np

from concourse import bacc, bass, mybir, tile
from concourse.bass_utils import run_bass_kernel_spmd
from concourse.masks import make_identity

F32 = mybir.dt.float32
BF16 = mybir.dt.bfloat16
I32 = mybir.dt.int32

N_CORES = 8
P = 128          # partitions
NEG_SLOPE = 0.2
SENT_ASN = -60.0  # sentinel row attention logit => exp(lrelu(...)) ~ 6e-6


# ----------------------------------------------------------------------------
# Host-side graph plan (pure integer / layout work)
# ----------------------------------------------------------------------------

class Plan:
    pass


def build_plan(edge_index, n_nodes):
    """Shard edges by dst, degree-sort nodes per core, build gather offsets."""
    assert n_nodes % N_CORES == 0
    shard = n_nodes // N_CORES
    nb = -(-shard // P)           # node batches per core
    pos_n = nb * P                # padded positions per core

    # self-loops are handled on-chip from resident per-node tiles; the slab
    # only carries the real edges (saves ~6% of the indirect-DMA columns).
    src_all = edge_index[0].astype(np.int64)
    dst_all = edge_index[1].astype(np.int64)

    owner = dst_all // shard

    cores = []
    for c in range(N_CORES):
        m = owner == c
        es = src_all[m]
        ed = dst_all[m] - c * shard
        o = np.argsort(ed, kind="stable")
        es, ed = es[o], ed[o]
        deg = np.bincount(ed, minlength=shard)
        perm = np.argsort(-deg, kind="stable")  # descending degree
        cores.append((es, ed, deg, perm))

    # shared per-batch L schedule (max over cores of per-batch max degree)
    L = np.zeros(nb, dtype=np.int64)
    for es, ed, deg, perm in cores:
        pd = np.zeros(pos_n, dtype=np.int64)
        pd[:shard] = deg[perm]
        L = np.maximum(L, pd.reshape(nb, P).max(axis=1))
    L = np.maximum(L, 1)
    cum = np.concatenate([[0], np.cumsum(L)])
    SL = int(cum[-1])

    # global position map for the shared layer-2 table. The all-gather is
    # issued as TWO uneven row-range chunks (each concatenated core-major):
    # a big chunk fired as soon as its rows are produced, and a small tail
    # chunk covering the last batches, so the inter-layer wait is short.
    split_b = max(1, nb - 4)
    Q0 = split_b * P
    Q1 = pos_n - Q0
    posmap = np.empty(n_nodes, dtype=np.int64)
    for c, (es, ed, deg, perm) in enumerate(cores):
        q = np.arange(shard)
        posmap[c * shard + perm] = np.where(
            q < Q0, c * Q0 + q, N_CORES * Q0 + c * Q1 + (q - Q0))

    sent1 = N_CORES * pos_n        # sentinel row in table1
    sent2 = N_CORES * pos_n        # sentinel row in table2

    # Layer-1 tables are PRIVATE per core (h = x@W1 is recomputed redundantly
    # for all nodes on every core — no collective). Each core's t1f is laid
    # out in its own staging order: own shard's batches first (so the
    # SPMD-static resident slots 0..NB-1 are the core's own nodes), then the
    # peers' shards in ascending core order.
    slotmaps = []
    for c in range(N_CORES):
        sm = np.empty(n_nodes, dtype=np.int64)
        order = [c] + [cc for cc in range(N_CORES) if cc != c]
        for rank, cc in enumerate(order):
            perm_ = cores[cc][3]
            sm[cc * shard + perm_] = rank * pos_n + np.arange(shard)
        slotmaps.append(sm)

    pc = []
    for c, (es, ed, deg, perm) in enumerate(cores):
        qn = np.empty(shard, dtype=np.int64)
        qn[perm] = np.arange(shard)               # local node -> position
        row_start = np.concatenate([[0], np.cumsum(deg)])
        q = qn[ed]                                 # position of each edge's dst
        j = np.arange(len(ed)) - row_start[ed]     # rank within node
        b = q // P
        p = q % P
        col = cum[b] + j

        offs1 = np.full((P, SL), sent1, dtype=np.int32)
        offs1[p, col] = slotmaps[c][es]            # layer 1: private t1f order
        offs2 = np.full((P, SL), sent2, dtype=np.int32)
        offs2[p, col] = posmap[es]                 # layer 2: shared t2f order

        pl = Plan()
        pl.offs1, pl.offs2, pl.perm = offs1, offs2, perm
        pc.append(pl)

    plan = Plan()
    plan.shard, plan.nb, plan.pos_n, plan.L, plan.cum, plan.SL = \
        shard, nb, pos_n, L.astype(int), cum.astype(int), SL
    plan.sent1, plan.sent2 = sent1, sent2
    plan.ag_chunks = [(0, Q0, 0), (Q0, pos_n, N_CORES * Q0)]
    plan.split_b = split_b
    plan.cores = pc
    return plan


def _chunks(L, max_sl, max_nb=12):
    """Split batch indices into chunks with bounded total L.

    The final chunk is kept tiny (<=2 batches) so the layer's last
    gather->compute->store tail — which gates the next phase via the last
    AllGather chunk — is as short as possible.
    """
    out = []
    b = 0
    nb = len(L)
    while b < nb:
        e = b
        s = 0
        while e < nb and e - b < max_nb and s + L[e] <= max_sl:
            s += L[e]
            e += 1
        if e == b:  # single oversized batch
            e = b + 1
        out.append((b, e))
        b = e
    if len(out) >= 1 and out[-1][1] - out[-1][0] > 2:
        b0, b1 = out[-1]
        out[-1] = (b0, b1 - 2)
        out.append((b1 - 2, b1))
    return out


# ----------------------------------------------------------------------------
# Device program
# ----------------------------------------------------------------------------

def build_program(plan, f_in, h1, c1, ncls):
    n_nodes = plan.shard * N_CORES
    F1 = h1 * c1                  # layer-1 hidden width (128)
    R1 = F1 + h1                  # table1 row: [h | asn]
    R2 = ncls + 2                 # table2 row: [h2' | asn2 | adn2]
    NB, SL, POS = plan.nb, plan.SL, plan.pos_n

    nc = bacc.Bacc(target_bir_lowering=False, debug=False, num_devices=N_CORES)

    # kernel I/O (xT is the FULL node set in this core's t1f staging order)
    xT = nc.declare_dram_parameter("xT", [f_in, N_CORES * POS], BF16,
                                   isOutput=False)
    W1 = nc.declare_dram_parameter("W1", [f_in, F1], F32, isOutput=False)
    W1T = nc.declare_dram_parameter("W1T", [F1, f_in], F32, isOutput=False)
    AB1 = nc.declare_dram_parameter("AB1", [F1, 2 * h1], F32, isOutput=False)
    b1r = nc.declare_dram_parameter("b1r", [1, F1], F32, isOutput=False)
    W2 = nc.declare_dram_parameter("W2", [F1, ncls], F32, isOutput=False)
    W2T = nc.declare_dram_parameter("W2T", [ncls, F1], F32, isOutput=False)
    A2 = nc.declare_dram_parameter("A2", [ncls, 2], F32, isOutput=False)
    b2r = nc.declare_dram_parameter("b2r", [1, ncls], F32, isOutput=False)
    offs1D = nc.declare_dram_parameter("offs1", [P, SL], I32, isOutput=False)
    offs2D = nc.declare_dram_parameter("offs2", [P, SL], I32, isOutput=False)
    outD = nc.declare_dram_parameter("out", [POS, ncls], F32, isOutput=True)

    # internal DRAM (t1f is private: every core computes the full table)
    t1f = nc.dram_tensor("t1full", [N_CORES * POS + 1, R1], BF16)
    t2s = nc.dram_tensor("t2shard", [POS, R2], BF16)
    t2f = nc.dram_tensor("t2full", [N_CORES * POS + 1, R2], BF16,
                         addr_space="Shared")

    rg = [list(range(N_CORES))]
    # fire the big AG2 chunk a couple of batches after its rows complete
    # (so the Pool queue never waits on the compute chain), and the tail
    # chunk after the last batch.
    ag2_fire = [min(plan.split_b + 2, NB - 2), NB - 1]

    from contextlib import ExitStack
    with tile.TileContext(nc) as tc, ExitStack() as ctx:
        const = ctx.enter_context(tc.tile_pool(name="const", bufs=1))
        resid = ctx.enter_context(tc.tile_pool(name="resid", bufs=1))
        sb = ctx.enter_context(tc.tile_pool(name="sb", bufs=3))
        big = ctx.enter_context(tc.tile_pool(name="big", bufs=2))
        psc = ctx.enter_context(tc.tile_pool(name="psc", bufs=1, space="PSUM"))
        ps = ctx.enter_context(tc.tile_pool(name="ps", bufs=2, space="PSUM"))

        # ------------------- phase 0: constants -------------------
        W1sb = const.tile([f_in, F1], F32)
        nc.sync.dma_start(W1sb[:], W1[:, :])
        W1Tsb = const.tile([F1, f_in], F32)
        nc.sync.dma_start(W1Tsb[:], W1T[:, :])
        AB1sb = const.tile([F1, 2 * h1], F32)
        nc.sync.dma_start(AB1sb[:], AB1[:, :])
        W2sb = const.tile([F1, ncls], F32)
        nc.sync.dma_start(W2sb[:], W2[:, :])
        W2Tsb = const.tile([ncls, F1], F32)
        nc.sync.dma_start(W2Tsb[:], W2T[:, :])
        A2sb = const.tile([ncls, 2], F32)
        nc.sync.dma_start(A2sb[:], A2[:, :])
        b1row = const.tile([1, F1], F32)
        nc.sync.dma_start(b1row[:], b1r[:, :])
        b2row = const.tile([1, ncls], F32)
        nc.sync.dma_start(b2row[:], b2r[:, :])

        # W1aug = [W1 | W1@Asrc | W1@Adst]  (fp32, rhs of phase-1 matmuls)
        psA = psc.tile([f_in, 2 * h1], F32, tag="ps_small")
        nc.tensor.matmul(psA[:], lhsT=W1Tsb[:], rhs=AB1sb[:], start=True,
                         stop=True)
        W1aug = const.tile([f_in, F1 + 2 * h1], BF16)
        nc.vector.tensor_copy(W1aug[:, 0:F1], W1sb[:])
        nc.vector.tensor_copy(W1aug[:, F1:F1 + 2 * h1], psA[:])

        # W2aug = [W2 | W2@a_src2 | W2@a_dst2]  (bf16)
        psB = psc.tile([F1, 2], F32, tag="ps_small")
        nc.tensor.matmul(psB[:], lhsT=W2Tsb[:], rhs=A2sb[:], start=True,
                         stop=True)
        W2aug = const.tile([F1, R2], BF16)
        nc.vector.tensor_copy(W2aug[:, 0:ncls], W2sb[:])
        nc.vector.tensor_copy(W2aug[:, ncls:R2], psB[:])

        # bias rows broadcast to all partitions (ones ⊗ row via K=1 matmul)
        ones1 = const.tile([1, P], F32)
        nc.vector.memset(ones1[:], 1.0)
        psb1 = psc.tile([P, F1], F32, tag="ps_bias")
        nc.tensor.matmul(psb1[:], lhsT=ones1[:], rhs=b1row[:], start=True,
                         stop=True)
        b1bc = const.tile([P, F1], F32)
        nc.vector.tensor_copy(b1bc[:], psb1[:])
        psb2 = psc.tile([P, ncls], F32, tag="ps_bias")
        nc.tensor.matmul(psb2[:], lhsT=ones1[:], rhs=b2row[:], start=True,
                         stop=True)
        b2bc = const.tile([P, ncls], F32)
        nc.vector.tensor_copy(b2bc[:], psb2[:])

        ident = const.tile([P, P], BF16)
        make_identity(nc, ident[:])

        # sentinel rows
        s1 = const.tile([1, R1], BF16)
        nc.vector.memset(s1[:, 0:F1], 0.0)
        nc.vector.memset(s1[:, F1:R1], SENT_ASN)
        nc.sync.dma_start(t1f[N_CORES * POS:N_CORES * POS + 1, :], s1[:])
        s2 = const.tile([1, R2], BF16)
        nc.vector.memset(s2[:, 0:ncls], 0.0)
        nc.vector.memset(s2[:, ncls:R2], SENT_ASN)
        nc.sync.dma_start(t2f[N_CORES * POS:N_CORES * POS + 1, :], s2[:])

        # per-node tiles kept resident in SBUF for the self-loop terms and
        # the dst-side attention values (no DRAM roundtrip)
        nR = resid.tile([P, NB, R1 + h1], BF16)   # [h | asn | adn] per node
        hR = nR[:, :, 0:F1]
        asnR = nR[:, :, F1:F1 + h1]
        adnR = nR[:, :, R1:R1 + h1]
        h2R = resid.tile([P, NB, ncls], BF16)
        asn2R = resid.tile([P, NB], BF16)
        adn2sb = resid.tile([P, NB], BF16)

        def ag(table_s, table_f, s, e, ob):
            # Collectives must issue from gpsimd (PE/ACT-queue issue faults
            # the HW runtime) and occupy the queue for the whole transfer,
            # so the chunking below minimizes total Pool-queue occupancy:
            # one big early chunk + one small tail chunk.
            nc.gpsimd.collective_compute(
                "AllGather", mybir.AluOpType.bypass, replica_groups=rg,
                ins=[table_s[s:e, :].opt()],
                outs=[table_f[ob:ob + N_CORES * (e - s), :].opt()])

        # ---- phase 1: h1 = x @ W1 for ALL nodes (redundant on every core;
        # cheaper than putting an AllGather of the table on the critical
        # path). Slots 0..NB-1 are this core's own nodes -> residents too.
        GB = 4                 # batches per DMA group (amortize HWDGE cost)
        assert (N_CORES * NB) % GB == 0
        for t0 in range(0, N_CORES * NB, GB):
            xt = sb.tile([f_in, GB * P], BF16, tag="xt")
            nc.sync.dma_start(xt[:], xT[:, t0 * P:(t0 + GB) * P])
            rows = sb.tile([P, GB, R1], BF16, tag="rows")
            for j in range(GB):
                t = t0 + j
                p1 = ps.tile([P, F1 + 2 * h1], F32, tag="ps_p1")
                nc.tensor.matmul(p1[:], lhsT=xt[:, j * P:(j + 1) * P],
                                 rhs=W1aug[:], start=True, stop=True)
                nc.vector.tensor_copy(rows[:, j, :], p1[:, 0:R1])
                if t < NB:
                    nc.vector.tensor_copy(nR[:, t, :], p1[:, 0:R1 + h1])
            nc.scalar.dma_start(
                t1f[t0 * P:(t0 + GB) * P, :].rearrange("(b p) r -> p b r",
                                                       p=P),
                rows[:])

        # ------------------- edge phases -------------------
        def edge_phase(layer):
            if layer == 1:
                table, offsD, R, F, H = t1f, offs1D, R1, F1, h1
                max_sl = 112
            else:
                table, offsD, R, F, H = t2f, offs2D, R2, ncls, 1
                max_sl = 160
            C = F // H

            for (b0, b1_) in _chunks(plan.L, max_sl):
                c0, c1_ = plan.cum[b0], plan.cum[b1_]
                slc = int(c1_ - c0)
                nbc = b1_ - b0

                osb = sb.tile([P, slc], I32, tag=f"osb{layer}")
                nc.sync.dma_start(osb[:], offsD[:, c0:c1_])
                # one [128,1] indirect gather per slot column (the only form
                # the HW DGE lowers correctly)
                slab = big.tile([P, slc, R], BF16, tag=f"slab{layer}")
                for j in range(slc):
                    nc.gpsimd.indirect_dma_start(
                        out=slab[:, j, :], out_offset=None, in_=table[:, :],
                        in_offset=bass.IndirectOffsetOnAxis(
                            ap=osb[:, j:j + 1], axis=0))

                # self-loop attention term for the chunk's own nodes:
                # eeS = exp(lrelu(asn + adn))
                if layer == 1:
                    asn_c = asnR[:, b0:b1_, :]
                    adn_c = adnR[:, b0:b1_, :]
                else:
                    asn_c = asn2R[:, b0:b1_].unsqueeze(2)
                    adn_c = adn2sb[:, b0:b1_].unsqueeze(2)
                eeS = sb.tile([P, nbc, H], F32, tag=f"eeS{layer}")
                nc.vector.tensor_tensor(out=eeS[:], in0=asn_c, in1=adn_c,
                                        op=mybir.AluOpType.add)
                nc.vector.scalar_tensor_tensor(
                    out=eeS[:], in0=eeS[:], scalar=NEG_SLOPE, in1=eeS[:],
                    op0=mybir.AluOpType.mult, op1=mybir.AluOpType.max)
                nc.scalar.activation(eeS[:], eeS[:],
                                     mybir.ActivationFunctionType.Exp)

                for bi in range(nbc):
                    b = b0 + bi
                    L = int(plan.L[b])
                    o = int(plan.cum[b] - c0)
                    sv = slab[:, o:o + L, :]

                    # e = lrelu(asn[src] + adn[dst])
                    e = sb.tile([P, L, H], F32, tag=f"e{layer}")
                    if layer == 1:
                        adn_b = adnR[:, b:b + 1, :].broadcast_to([P, L, H])
                    else:
                        adn_b = adn2sb[:, b:b + 1].unsqueeze(2) \
                            .broadcast_to([P, L, H])
                    nc.vector.tensor_tensor(
                        out=e[:], in0=sv[:, :, F:F + H], in1=adn_b,
                        op=mybir.AluOpType.add)
                    # leaky-relu: e = max(0.2*e, e)
                    nc.vector.scalar_tensor_tensor(
                        out=e[:], in0=e[:], scalar=NEG_SLOPE, in1=e[:],
                        op0=mybir.AluOpType.mult, op1=mybir.AluOpType.max)
                    ee = sb.tile([P, L, H], BF16, tag=f"ee{layer}")
                    nc.scalar.activation(ee[:], e[:],
                                         mybir.ActivationFunctionType.Exp)

                    # m[p, f, j] = h[p, j, f] * ee[p, j, head(f)]
                    m = big.tile([P, F, L], BF16, tag=f"m{layer}")
                    if H > 1:
                        m_v = m[:].rearrange("p (h c) l -> p h c l", h=H)
                        h_v = sv[:, :, 0:F].rearrange("p l (h c) -> p h c l",
                                                      h=H)
                        ee_v = ee[:].rearrange("p l h -> p h l").unsqueeze(2) \
                            .broadcast_to([P, H, C, L])
                    else:
                        m_v = m[:]
                        h_v = sv[:, :, 0:F].rearrange("p l c -> p c l")
                        ee_v = ee[:].rearrange("p l h -> p h l") \
                            .broadcast_to([P, C, L])
                    nc.any.tensor_tensor(out=m_v, in0=h_v, in1=ee_v,
                                         op=mybir.AluOpType.mult)

                    msg = sb.tile([P, F], F32, tag=f"msg{layer}")
                    nc.vector.tensor_reduce(out=msg[:], in_=m[:],
                                            axis=mybir.AxisListType.X,
                                            op=mybir.AluOpType.add)
                    den = sb.tile([P, H], F32, tag=f"den{layer}")
                    nc.vector.tensor_reduce(
                        out=den[:], in_=ee[:].rearrange("p l h -> p h l"),
                        axis=mybir.AxisListType.X, op=mybir.AluOpType.add)

                    # self-loop contribution (node's own h, weight eeS)
                    nc.vector.tensor_tensor(out=den[:], in0=den[:],
                                            in1=eeS[:, bi, :],
                                            op=mybir.AluOpType.add)
                    ms = sb.tile([P, F], F32, tag=f"ms{layer}")
                    hsrc = hR[:, b, :] if layer == 1 else h2R[:, b, :]
                    nc.vector.tensor_tensor(
                        out=ms[:].rearrange("p (h c) -> p h c", h=H),
                        in0=hsrc.rearrange("p (h c) -> p h c", h=H),
                        in1=eeS[:, bi, :].unsqueeze(2).broadcast_to([P, H, C]),
                        op=mybir.AluOpType.mult)
                    nc.vector.tensor_tensor(out=msg[:], in0=msg[:], in1=ms[:],
                                            op=mybir.AluOpType.add)

                    rec = sb.tile([P, H], F32, tag=f"rec{layer}")
                    nc.vector.reciprocal(rec[:], den[:])

                    # out = msg / den + bias
                    o1 = sb.tile([P, F], F32, tag=f"o1_{layer}")
                    nc.vector.tensor_tensor(
                        out=o1[:].rearrange("p (h c) -> p h c", h=H),
                        in0=msg[:].rearrange("p (h c) -> p h c", h=H),
                        in1=rec[:].unsqueeze(2).broadcast_to([P, H, C]),
                        op=mybir.AluOpType.mult)
                    bias = b1bc if layer == 1 else b2bc
                    nc.vector.tensor_tensor(out=o1[:], in0=o1[:], in1=bias[:],
                                            op=mybir.AluOpType.add)

                    if layer == 1:
                        # h2 = elu(o1) = relu(o1) + min(exp(o1), 1) - 1
                        t1_ = sb.tile([P, F], F32, tag="elu1")
                        nc.scalar.activation(t1_[:], o1[:],
                                             mybir.ActivationFunctionType.Exp)
                        nc.vector.tensor_scalar_min(t1_[:], t1_[:], 1.0)
                        t2_ = sb.tile([P, F], F32, tag="elu2")
                        nc.scalar.activation(t2_[:], o1[:],
                                             mybir.ActivationFunctionType.Relu)
                        nc.vector.tensor_tensor(out=t1_[:], in0=t1_[:],
                                                in1=t2_[:],
                                                op=mybir.AluOpType.add)
                        h2 = sb.tile([P, F], BF16, tag="h2")
                        nc.vector.tensor_scalar_add(h2[:], t1_[:], -1.0)

                        # h2' = h2 @ W2aug  (via PE transpose of h2)
                        pst = ps.tile([P, P], BF16, tag="ps_t")
                        nc.tensor.transpose(pst[:], h2[:], ident[:])
                        h2T = sb.tile([P, P], BF16, tag="h2T")
                        nc.vector.tensor_copy(h2T[:], pst[:])
                        p2 = ps.tile([P, R2], F32, tag="ps_2")
                        nc.tensor.matmul(p2[:], lhsT=h2T[:], rhs=W2aug[:],
                                         start=True, stop=True)
                        tw = sb.tile([P, R2], BF16, tag="tw")
                        nc.vector.tensor_copy(tw[:], p2[:])
                        nc.vector.tensor_copy(h2R[:, b, :], p2[:, 0:ncls])
                        nc.vector.tensor_copy(asn2R[:, b:b + 1],
                                              p2[:, ncls:ncls + 1])
                        nc.vector.tensor_copy(adn2sb[:, b:b + 1],
                                              p2[:, ncls + 1:ncls + 2])
                        nc.scalar.dma_start(t2s[b * P:(b + 1) * P, :], tw[:])
                        for k, (s_, e_, ob_) in enumerate(plan.ag_chunks):
                            if ag2_fire[k] == b:
                                ag(t2s, t2f, s_, e_, ob_)
                    else:
                        # log_softmax
                        ex = sb.tile([P, F], F32, tag="lsm_e")
                        s = sb.tile([P, 1], F32, tag="lsm_s")
                        nc.scalar.activation(ex[:], o1[:],
                                             mybir.ActivationFunctionType.Exp,
                                             accum_out=s[:])
                        ln = sb.tile([P, 1], F32, tag="lsm_l")
                        nc.scalar.activation(ln[:], s[:],
                                             mybir.ActivationFunctionType.Ln)
                        fo = sb.tile([P, F], F32, tag="fo")
                        nc.vector.tensor_tensor(
                            out=fo[:], in0=o1[:],
                            in1=ln[:].broadcast_to([P, F]),
                            op=mybir.AluOpType.subtract)
                        nc.scalar.dma_start(outD[b * P:(b + 1) * P, :], fo[:])

        edge_phase(1)
        edge_phase(2)

    nc.compile()
    return nc


# ----------------------------------------------------------------------------
# Entry point
# ----------------------------------------------------------------------------

def _block_diag_a(a_src, a_dst):
    h, c = a_src.shape
    F1 = h * c
    ab = np.zeros((F1, 2 * h), dtype=np.float32)
    for hd in range(h):
        ab[hd * c:(hd + 1) * c, hd] = a_src[hd]
        ab[hd * c:(hd + 1) * c, h + hd] = a_dst[hd]
    return ab


def prepare(x, edge_index, W1, a_src1, a_dst1, b1, W2, a_src2, a_dst2, b2):
    x = np.asarray(x, dtype=np.float32)
    edge_index = np.asarray(edge_index)
    n_nodes, f_in = x.shape
    h1, c1 = np.asarray(a_src1).shape
    ncls = np.asarray(W2).shape[1]

    plan = build_plan(edge_index, n_nodes)
    nc = build_program(plan, f_in, h1, c1, ncls)

    AB1 = _block_diag_a(np.asarray(a_src1, np.float32),
                        np.asarray(a_dst1, np.float32))
    A2 = np.concatenate([np.asarray(a_src2, np.float32).T,
                         np.asarray(a_dst2, np.float32).T], axis=1)
    common = {
        "W1": np.ascontiguousarray(W1, np.float32),
        "W1T": np.ascontiguousarray(np.asarray(W1, np.float32).T),
        "AB1": AB1,
        "b1r": np.asarray(b1, np.float32).reshape(1, -1),
        "W2": np.ascontiguousarray(W2, np.float32),
        "W2T": np.ascontiguousarray(np.asarray(W2, np.float32).T),
        "A2": np.ascontiguousarray(A2),
        "b2r": np.asarray(b2, np.float32).reshape(1, -1),
    }
    import ml_dtypes
    pad = np.zeros((plan.pos_n - plan.shard, f_in), np.float32)
    shards = []           # per source core: batch-permuted, padded shard
    for c in range(N_CORES):
        xs = x[c * plan.shard:(c + 1) * plan.shard][plan.cores[c].perm]
        shards.append(np.concatenate([xs, pad]))
    in_maps = []
    for c in range(N_CORES):
        pl = plan.cores[c]
        im = dict(common)
        order = [c] + [cc for cc in range(N_CORES) if cc != c]
        xall = np.concatenate([shards[cc] for cc in order])
        im["xT"] = np.ascontiguousarray(xall.T).astype(ml_dtypes.bfloat16)
        im["offs1"] = pl.offs1
        im["offs2"] = pl.offs2
        in_maps.append(im)
    return plan, nc, in_maps, (n_nodes, ncls)


def finish(plan, shard_outs, n_nodes, ncls):
    out = np.empty((n_nodes, ncls), dtype=np.float32)
    for c in range(N_CORES):
        pl = plan.cores[c]
        out[c * plan.shard + pl.perm] = shard_outs[c][:plan.shard]
    return out


def kernel(x, edge_index, W1, a_src1, a_dst1, b1, W2, a_src2, a_dst2, b2,
           **run_kwargs):
    plan, nc, in_maps, (n_nodes, ncls) = prepare(
        x, edge_index, W1, a_src1, a_dst1, b1, W2, a_src2, a_dst2, b2)
    res = run_bass_kernel_spmd(nc, in_maps, core_ids=list(range(N_CORES)),
                               **run_kwargs)
    out = finish(plan, [res.results[c]["out"] for c in range(N_CORES)],
                 n_nodes, ncls)
    kernel.last_result = res
    return out

